# revision 17
# baseline (speedup 1.0000x reference)
"""Expert-parallel MoE layer for one TRN2 chip (8 NeuronCores).

Problem: B=2, S=2048, D=1024, E=8, H=4*D, TOP_K=2 (nn_MoELayer, moe_routing).

Sharding strategy (expert-parallel, "all-to-all at shard time"):
  * The gate is tiny ([4096,1024]@[1024,8]) and runs on host in float64 while
    sharding.  float64 routing reproduces the fp32 reference's top-2 exactly:
    the minimum 2nd/3rd logit gap for these inputs is ~4.4e-4 while fp32
    accumulation noise is ~2e-6.
  * Expert e's (w1, b1, w2) are placed on core e.  The tokens routed to
    expert e (~1024 of 4096*2 slots, capacity CAP with zero padding) are
    gathered on host and shipped pre-transposed as xT[D, CAP].
  * Each core computes yT = w2 @ gelu(w1 @ xT + b1) with bf16 matmuls and
    fp32 PSUM accumulation; layouts are chosen so no on-device transpose is
    ever needed (mm1 produces h already contracted-major for mm2).
  * The softmax-weighted combine (+ per-expert b2) happens on host during the
    unshard/gather step.  Tokens above CAP (never for these shapes) fall back
    to an exact host computation.
"""

import math
import os

import numpy as np
import ml_dtypes

import concourse.bacc as bacc
import concourse.mybir as mybir
import concourse.tile as tile
from concourse import bass_utils
from concourse.bass import ds, ts
from concourse._compat import get_trn_type

P = 128
B, S, D, E, TOP_K = 2, 2048, 1024, 8, 2
H = 4 * D
T = B * S
CAP = 1088   # per-expert token capacity (max routed count is 1088 for these inputs)
CHUNK = 272  # token chunk per PSUM group; >230 keeps 97ns LDWEIGHTS hidden under the MM stream

BF16 = mybir.dt.bfloat16
F32 = mybir.dt.float32

GELU_FUNC = mybir.ActivationFunctionType.Gelu  # sim tests may swap (CoreSim lacks Gelu)

_BUILD_CACHE = {}
LAST_RESULTS = None  # BassKernelResults of the most recent device run (for profiling)


def _build(cap=CAP, h=H, d=D, chunk=CHUNK):
    """Build + compile the per-core Bass program (SPMD, one expert per core)."""
    dt_tiles, ht_tiles = d // P, h // P
    nc = bacc.Bacc(get_trn_type() or "TRN2", target_bir_lowering=False, debug=False)

    n_chunks = math.ceil(cap / chunk)
    # x and w1 are laid out in DRAM grouped by the DMA block that fetches
    # them (chunk-major for x, column-block-major for w1) so every DMA reads
    # long contiguous per-partition lines (>=2KB) at full HBM bandwidth while
    # still letting the first blocks arrive first.  w1's first 512 columns
    # ship as four 128-col blocks so the PE can start ~3us earlier; the rest
    # as 256-col blocks.
    n_w1a = min(4, h // P)
    n_w1b = (h - n_w1a * P) // 256
    x_d = nc.dram_tensor("xt", [P, n_chunks, dt_tiles, chunk], BF16, kind="ExternalInput")
    w1a_d = nc.dram_tensor("w1a", [P, n_w1a, dt_tiles, P], BF16, kind="ExternalInput")
    w1b_d = (
        nc.dram_tensor("w1b", [P, n_w1b, dt_tiles, 256], BF16, kind="ExternalInput")
        if n_w1b
        else None
    )
    w2_d = nc.dram_tensor("w2t", [P, ht_tiles, d], BF16, kind="ExternalInput")
    b1_d = nc.dram_tensor("b1p", [P, ht_tiles], F32, kind="ExternalInput")
    y_d = nc.dram_tensor("yt", [P, dt_tiles, cap], F32, kind="ExternalOutput")

    with tile.TileContext(nc) as tc:
        with (
            tc.tile_pool(name="wts", bufs=1) as wpool,
            tc.tile_pool(name="io", bufs=1) as ypool,
            tc.tile_pool(name="hts", bufs=2) as htpool,
            tc.tile_pool(name="ps1", bufs=4, space="PSUM") as ps1,
            tc.tile_pool(name="ps2", bufs=4, space="PSUM") as ps2,
        ):
            # DMA order shapes the pipeline lead-in: the first matmuls only
            # need x[chunk 0] plus w1's first column-block, so ship x chunk 0,
            # then w1 in column-blocks, then x chunk 1, then w2.  HBM can't
            # deliver w1+w2 (~17MB) inside one mm1 chunk, so the compute
            # below runs mm1(c0), mm1(c1) before mm2(c0) to buy the w2
            # transfer time.
            x_sb = wpool.tile([P, dt_tiles, cap], BF16)
            w1_sb = wpool.tile([P, dt_tiles, h], BF16)
            w2_sb = wpool.tile([P, ht_tiles, d], BF16)
            b1_sb = wpool.tile([P, ht_tiles], F32)

            def dma_x_chunk(ci):
                c0 = ci * chunk
                n = min(chunk, cap - c0)
                nc.sync.dma_start(
                    out=x_sb[:, :, ds(c0, n)], in_=x_d[:, ci, :, :n]
                )

            nc.sync.dma_start(out=b1_sb[:], in_=b1_d[:])
            # x chunk 0 in two k-halves so the first (m=0, k<half) matmuls
            # can issue before the whole chunk lands
            kh = max(1, dt_tiles // 2)
            nc.sync.dma_start(out=x_sb[:, :kh, ds(0, chunk)], in_=x_d[:, 0, :kh, :])
            nc.sync.dma_start(out=x_sb[:, kh:, ds(0, chunk)], in_=x_d[:, 0, kh:, :])
            for mb in range(n_w1a):
                nc.sync.dma_start(
                    out=w1_sb[:, :, ts(mb, P)], in_=w1a_d[:, mb, :, :]
                )
            for jb in range(n_w1b):
                nc.sync.dma_start(
                    out=w1_sb[:, :, ds(n_w1a * P + jb * 256, 256)],
                    in_=w1b_d[:, jb, :, :],
                )
            if n_chunks > 1:
                dma_x_chunk(1)
            for k in range(ht_tiles):
                nc.sync.dma_start(out=w2_sb[:, k, :], in_=w2_d[:, k, :])
            for ci in range(2, n_chunks):
                dma_x_chunk(ci)

            ht_tiles_sb = {}

            def mm1_chunk(ci):
                c0 = ci * chunk
                n = min(chunk, cap - c0)
                # mm1: hT[h_tile, tok] = sum_d w1t[d, h_tile] * xT[d, tok]
                ht_sb = htpool.tile([P, ht_tiles, chunk], BF16, tag="ht")
                ht_tiles_sb[ci] = ht_sb
                for m in range(ht_tiles):
                    ps = ps1.tile([P, chunk], F32, tag="ps1")
                    for k in range(dt_tiles):
                        nc.tensor.matmul(
                            ps[:, :n],
                            w1_sb[:, k, ts(m, P)],
                            x_sb[:, k, ds(c0, n)],
                            start=(k == 0),
                            stop=(k == dt_tiles - 1),
                        )
                    nc.scalar.activation(
                        ht_sb[:, m, :n],
                        ps[:, :n],
                        GELU_FUNC,
                        bias=b1_sb[:, m : m + 1],
                    )

            def mm2_chunk(ci):
                c0 = ci * chunk
                n = min(chunk, cap - c0)
                ht_sb = ht_tiles_sb.pop(ci)
                # mm2: yT[d_tile, tok] = sum_h w2t[h, d_tile] * hT[h, tok]
                y_sb = ypool.tile([P, dt_tiles, chunk], F32, tag="y")
                for m in range(dt_tiles):
                    ps = ps2.tile([P, chunk], F32, tag="ps2")
                    for k in range(ht_tiles):
                        nc.tensor.matmul(
                            ps[:, :n],
                            w2_sb[:, k, ts(m, P)],
                            ht_sb[:, k, :n],
                            start=(k == 0),
                            stop=(k == ht_tiles - 1),
                        )
                    nc.vector.tensor_copy(y_sb[:, m, :n], ps[:, :n])
                    nc.sync.dma_start(out=y_d[:, m, ds(c0, n)], in_=y_sb[:, m, :n])

            # software pipeline: two mm1 chunks in flight ahead of mm2
            mm1_chunk(0)
            if n_chunks > 1:
                mm1_chunk(1)
            mm2_chunk(0)
            for ci in range(2, n_chunks):
                mm1_chunk(ci)
                mm2_chunk(ci - 1)
            if n_chunks > 1:
                mm2_chunk(n_chunks - 1)

    nc.compile()
    return nc


def _get_nc(cap=CAP, h=H, d=D, chunk=CHUNK):
    key = (cap, h, d, chunk)
    if key not in _BUILD_CACHE:
        _BUILD_CACHE[key] = _build(*key)
    return _BUILD_CACHE[key]


def _part3(mat_t):
    """[Dim, N] (Dim multiple of 128) -> partition-major [128, Dim/128, N]."""
    dim, n = mat_t.shape
    return np.ascontiguousarray(mat_t.reshape(dim // P, P, n).transpose(1, 0, 2))


def _gelu_exact(v):
    from scipy.special import erf

    return 0.5 * v * (1.0 + erf(v / np.sqrt(2.0)))


def kernel(x, gate_w, gate_b, w1, b1, w2, b2):
    global LAST_RESULTS
    xf = np.asarray(x, np.float32).reshape(T, D)
    gate_w = np.asarray(gate_w, np.float32)
    gate_b = np.asarray(gate_b, np.float32)
    w1 = np.asarray(w1, np.float32)
    b1 = np.asarray(b1, np.float32)
    w2 = np.asarray(w2, np.float32)
    b2 = np.asarray(b2, np.float32)

    # --- host gate + top-2 routing (float64: robust vs fp32 rounding) ---
    logits = xf.astype(np.float64) @ gate_w.T.astype(np.float64) + gate_b
    sel = np.argsort(-logits, axis=1, kind="stable")[:, :TOP_K].astype(np.int32)
    lsel = np.take_along_axis(logits, sel.astype(np.int64), axis=1)
    ex = np.exp(lsel - lsel.max(axis=1, keepdims=True))
    wts = (ex / ex.sum(axis=1, keepdims=True)).astype(np.float32)

    # --- gather tokens per expert (shard step) ---
    idx_e, wt_e, ovf = [], [], []
    in_maps = []
    xT = np.ascontiguousarray(xf.T)  # [D, T]
    for e in range(E):
        m0 = sel[:, 0] == e
        m1 = sel[:, 1] == e
        idx = np.nonzero(m0 | m1)[0]
        wt = np.where(m0[idx], wts[idx, 0], wts[idx, 1]).astype(np.float32)
        if len(idx) > CAP:
            for t_ov, w_ov in zip(idx[CAP:], wt[CAP:]):
                ovf.append((int(t_ov), e, float(w_ov)))
            idx, wt = idx[:CAP], wt[:CAP]
        idx_e.append(idx)
        wt_e.append(wt)

        n_chunks = CAP // CHUNK
        dt_tiles = D // P
        xt = np.zeros((P, dt_tiles, CAP), ml_dtypes.bfloat16)
        xt[:, :, : len(idx)] = _part3(xT[:, idx]).astype(ml_dtypes.bfloat16)
        # group by DMA block: x chunk-major, w1 column-block-major (contiguous lines)
        xt = np.ascontiguousarray(
            xt.reshape(P, dt_tiles, n_chunks, CHUNK).transpose(0, 2, 1, 3)
        )
        w1t = _part3(w1[e].T).astype(ml_dtypes.bfloat16)  # [P, dt_tiles, H]
        n_w1a = min(4, H // P)
        ca = n_w1a * P
        im = {
            "xt": xt,
            "w1a": np.ascontiguousarray(
                w1t[:, :, :ca].reshape(P, dt_tiles, n_w1a, P).transpose(0, 2, 1, 3)
            ),
            "w2t": _part3(w2[e].T).astype(ml_dtypes.bfloat16),
            "b1p": np.ascontiguousarray(b1[e].reshape(H // P, P).T),
        }
        if H > ca:
            im["w1b"] = np.ascontiguousarray(
                w1t[:, :, ca:]
                .reshape(P, dt_tiles, (H - ca) // 256, 256)
                .transpose(0, 2, 1, 3)
            )
        in_maps.append(im)

    # --- run the 8-core SPMD kernel ---
    nc = _get_nc(CAP, H, D, CHUNK)
    try:
        res = bass_utils.run_bass_kernel_spmd(nc, in_maps, core_ids=list(range(E)))
    except ImportError:
        # BASS_TRACE was requested but the axon NTFF hook isn't available.
        os.environ["BASS_NEVER_TRACE"] = "1"
        res = bass_utils.run_bass_kernel_spmd(nc, in_maps, core_ids=list(range(E)))
    LAST_RESULTS = res

    # --- weighted combine + b2 (unshard step) ---
    out = np.zeros((T, D), np.float32)
    for e in range(E):
        cnt = len(idx_e[e])
        yt = np.asarray(res.results[e]["yt"], np.float32)  # [128, D/128, CAP]
        y = yt.transpose(2, 1, 0).reshape(CAP, D)[:cnt]
        out[idx_e[e]] += wt_e[e][:, None] * (y + b2[e][None, :])

    for t_ov, e, w_ov in ovf:  # capacity overflow: exact host fallback
        hrow = _gelu_exact(xf[t_ov] @ w1[e].T + b1[e])
        out[t_ov] += w_ov * (hrow @ w2[e].T + b2[e])

    return out.reshape(B, S, D), sel.reshape(B, S, TOP_K)


# revision 18
# speedup vs baseline: 1.0317x; 1.0317x over previous
"""Expert-parallel MoE layer for one TRN2 chip (8 NeuronCores).

Problem: B=2, S=2048, D=1024, E=8, H=4*D, TOP_K=2 (nn_MoELayer, moe_routing).

Sharding strategy (expert-parallel, "all-to-all at shard time"):
  * The gate is tiny ([4096,1024]@[1024,8]) and runs on host in float64 while
    sharding.  float64 routing reproduces the fp32 reference's top-2 exactly:
    the minimum 2nd/3rd logit gap for these inputs is ~4.4e-4 while fp32
    accumulation noise is ~2e-6.
  * Expert e's (w1, b1, w2) are placed on core e.  The tokens routed to
    expert e (~1024 of 4096*2 slots, capacity CAP with zero padding) are
    gathered on host and shipped pre-transposed as xT[D, CAP].
  * Each core computes yT = w2 @ gelu(w1 @ xT + b1) with bf16 matmuls and
    fp32 PSUM accumulation; layouts are chosen so no on-device transpose is
    ever needed (mm1 produces h already contracted-major for mm2).
  * The softmax-weighted combine (+ per-expert b2) happens on host during the
    unshard/gather step.  Tokens above CAP (never for these shapes) fall back
    to an exact host computation.
"""

import math
import os

import numpy as np
import ml_dtypes

import concourse.bacc as bacc
import concourse.mybir as mybir
import concourse.tile as tile
from concourse import bass_utils
from concourse.bass import ds, ts
from concourse._compat import get_trn_type

P = 128
B, S, D, E, TOP_K = 2, 2048, 1024, 8, 2
H = 4 * D
T = B * S
CAP = 1088   # per-expert token capacity (max routed count is 1088 for these inputs)
CHUNK = 272  # token chunk per PSUM group; >230 keeps 97ns LDWEIGHTS hidden under the MM stream

BF16 = mybir.dt.bfloat16
F32 = mybir.dt.float32

GELU_FUNC = mybir.ActivationFunctionType.Gelu  # sim tests may swap (CoreSim lacks Gelu)

_BUILD_CACHE = {}
LAST_RESULTS = None  # BassKernelResults of the most recent device run (for profiling)


def _build(cap=CAP, h=H, d=D, chunk=CHUNK):
    """Build + compile the per-core Bass program (SPMD, one expert per core)."""
    dt_tiles, ht_tiles = d // P, h // P
    nc = bacc.Bacc(get_trn_type() or "TRN2", target_bir_lowering=False, debug=False)

    n_chunks = math.ceil(cap / chunk)
    # x and w1 are laid out in DRAM grouped by the DMA block that fetches
    # them (chunk-major for x, column-block-major for w1) so every DMA reads
    # long contiguous per-partition lines (>=2KB) at full HBM bandwidth while
    # still letting the first blocks arrive first.  w1's first 512 columns
    # ship as four 128-col blocks so the PE can start ~3us earlier; the rest
    # as 256-col blocks.
    n_w1a = min(4, h // P)
    n_w1b = (h - n_w1a * P) // 256
    x_d = nc.dram_tensor("xt", [P, n_chunks, dt_tiles, chunk], BF16, kind="ExternalInput")
    w1a_d = nc.dram_tensor("w1a", [P, n_w1a, dt_tiles, P], BF16, kind="ExternalInput")
    w1b_d = (
        nc.dram_tensor("w1b", [P, n_w1b, dt_tiles, 256], BF16, kind="ExternalInput")
        if n_w1b
        else None
    )
    w2_d = nc.dram_tensor("w2t", [P, ht_tiles, d], BF16, kind="ExternalInput")
    b1_d = nc.dram_tensor("b1p", [P, ht_tiles], F32, kind="ExternalInput")
    y_d = nc.dram_tensor("yt", [P, dt_tiles, cap], F32, kind="ExternalOutput")

    with tile.TileContext(nc) as tc:
        with (
            tc.tile_pool(name="wts", bufs=1) as wpool,
            tc.tile_pool(name="io", bufs=1) as ypool,
            tc.tile_pool(name="hts", bufs=2) as htpool,
            tc.tile_pool(name="ps1", bufs=4, space="PSUM") as ps1,
            tc.tile_pool(name="ps2", bufs=4, space="PSUM") as ps2,
        ):
            # DMA order shapes the pipeline lead-in: the first matmuls only
            # need x[chunk 0] plus w1's first column-block, so ship x chunk 0,
            # then w1 in column-blocks, then x chunk 1, then w2.  HBM can't
            # deliver w1+w2 (~17MB) inside one mm1 chunk, so the compute
            # below runs mm1(c0), mm1(c1) before mm2(c0) to buy the w2
            # transfer time.
            x_sb = wpool.tile([P, dt_tiles, cap], BF16)
            w1_sb = wpool.tile([P, dt_tiles, h], BF16)
            w2_sb = wpool.tile([P, ht_tiles, d], BF16)
            b1_sb = wpool.tile([P, ht_tiles], F32)

            def dma_x_chunk(ci):
                c0 = ci * chunk
                n = min(chunk, cap - c0)
                nc.sync.dma_start(
                    out=x_sb[:, :, ds(c0, n)], in_=x_d[:, ci, :, :n]
                )

            # front transfers go on GpSimd: its preamble clears ~1.4us before
            # Sync's first possible descriptor issue, and issuing there runs
            # in parallel with Sync issuing the w1 blocks below
            nc.gpsimd.dma_start(out=b1_sb[:], in_=b1_d[:])
            # x chunk 0 in two k-halves so the first (m=0, k<half) matmuls
            # can issue before the whole chunk lands
            kh = max(1, dt_tiles // 2)
            nc.gpsimd.dma_start(out=x_sb[:, :kh, ds(0, chunk)], in_=x_d[:, 0, :kh, :])
            nc.gpsimd.dma_start(out=x_sb[:, kh:, ds(0, chunk)], in_=x_d[:, 0, kh:, :])
            for mb in range(n_w1a):
                nc.sync.dma_start(
                    out=w1_sb[:, :, ts(mb, P)], in_=w1a_d[:, mb, :, :]
                )
            for jb in range(n_w1b):
                nc.sync.dma_start(
                    out=w1_sb[:, :, ds(n_w1a * P + jb * 256, 256)],
                    in_=w1b_d[:, jb, :, :],
                )
            if n_chunks > 1:
                dma_x_chunk(1)
            for k in range(ht_tiles):
                nc.sync.dma_start(out=w2_sb[:, k, :], in_=w2_d[:, k, :])
            for ci in range(2, n_chunks):
                dma_x_chunk(ci)

            ht_tiles_sb = {}

            def mm1_chunk(ci):
                c0 = ci * chunk
                n = min(chunk, cap - c0)
                # mm1: hT[h_tile, tok] = sum_d w1t[d, h_tile] * xT[d, tok]
                ht_sb = htpool.tile([P, ht_tiles, chunk], BF16, tag="ht")
                ht_tiles_sb[ci] = ht_sb
                for m in range(ht_tiles):
                    ps = ps1.tile([P, chunk], F32, tag="ps1")
                    for k in range(dt_tiles):
                        nc.tensor.matmul(
                            ps[:, :n],
                            w1_sb[:, k, ts(m, P)],
                            x_sb[:, k, ds(c0, n)],
                            start=(k == 0),
                            stop=(k == dt_tiles - 1),
                        )
                    nc.scalar.activation(
                        ht_sb[:, m, :n],
                        ps[:, :n],
                        GELU_FUNC,
                        bias=b1_sb[:, m : m + 1],
                    )

            def mm2_chunk(ci):
                c0 = ci * chunk
                n = min(chunk, cap - c0)
                ht_sb = ht_tiles_sb.pop(ci)
                # mm2: yT[d_tile, tok] = sum_h w2t[h, d_tile] * hT[h, tok]
                y_sb = ypool.tile([P, dt_tiles, chunk], F32, tag="y")
                for m in range(dt_tiles):
                    ps = ps2.tile([P, chunk], F32, tag="ps2")
                    for k in range(ht_tiles):
                        nc.tensor.matmul(
                            ps[:, :n],
                            w2_sb[:, k, ts(m, P)],
                            ht_sb[:, k, :n],
                            start=(k == 0),
                            stop=(k == ht_tiles - 1),
                        )
                    nc.vector.tensor_copy(y_sb[:, m, :n], ps[:, :n])
                    nc.sync.dma_start(out=y_d[:, m, ds(c0, n)], in_=y_sb[:, m, :n])

            # software pipeline: two mm1 chunks in flight ahead of mm2
            mm1_chunk(0)
            if n_chunks > 1:
                mm1_chunk(1)
            mm2_chunk(0)
            for ci in range(2, n_chunks):
                mm1_chunk(ci)
                mm2_chunk(ci - 1)
            if n_chunks > 1:
                mm2_chunk(n_chunks - 1)

    nc.compile()
    return nc


def _get_nc(cap=CAP, h=H, d=D, chunk=CHUNK):
    key = (cap, h, d, chunk)
    if key not in _BUILD_CACHE:
        _BUILD_CACHE[key] = _build(*key)
    return _BUILD_CACHE[key]


def _part3(mat_t):
    """[Dim, N] (Dim multiple of 128) -> partition-major [128, Dim/128, N]."""
    dim, n = mat_t.shape
    return np.ascontiguousarray(mat_t.reshape(dim // P, P, n).transpose(1, 0, 2))


def _gelu_exact(v):
    from scipy.special import erf

    return 0.5 * v * (1.0 + erf(v / np.sqrt(2.0)))


def kernel(x, gate_w, gate_b, w1, b1, w2, b2):
    global LAST_RESULTS
    xf = np.asarray(x, np.float32).reshape(T, D)
    gate_w = np.asarray(gate_w, np.float32)
    gate_b = np.asarray(gate_b, np.float32)
    w1 = np.asarray(w1, np.float32)
    b1 = np.asarray(b1, np.float32)
    w2 = np.asarray(w2, np.float32)
    b2 = np.asarray(b2, np.float32)

    # --- host gate + top-2 routing (float64: robust vs fp32 rounding) ---
    logits = xf.astype(np.float64) @ gate_w.T.astype(np.float64) + gate_b
    sel = np.argsort(-logits, axis=1, kind="stable")[:, :TOP_K].astype(np.int32)
    lsel = np.take_along_axis(logits, sel.astype(np.int64), axis=1)
    ex = np.exp(lsel - lsel.max(axis=1, keepdims=True))
    wts = (ex / ex.sum(axis=1, keepdims=True)).astype(np.float32)

    # --- gather tokens per expert (shard step) ---
    idx_e, wt_e, ovf = [], [], []
    in_maps = []
    xT = np.ascontiguousarray(xf.T)  # [D, T]
    for e in range(E):
        m0 = sel[:, 0] == e
        m1 = sel[:, 1] == e
        idx = np.nonzero(m0 | m1)[0]
        wt = np.where(m0[idx], wts[idx, 0], wts[idx, 1]).astype(np.float32)
        if len(idx) > CAP:
            for t_ov, w_ov in zip(idx[CAP:], wt[CAP:]):
                ovf.append((int(t_ov), e, float(w_ov)))
            idx, wt = idx[:CAP], wt[:CAP]
        idx_e.append(idx)
        wt_e.append(wt)

        n_chunks = CAP // CHUNK
        dt_tiles = D // P
        xt = np.zeros((P, dt_tiles, CAP), ml_dtypes.bfloat16)
        xt[:, :, : len(idx)] = _part3(xT[:, idx]).astype(ml_dtypes.bfloat16)
        # group by DMA block: x chunk-major, w1 column-block-major (contiguous lines)
        xt = np.ascontiguousarray(
            xt.reshape(P, dt_tiles, n_chunks, CHUNK).transpose(0, 2, 1, 3)
        )
        w1t = _part3(w1[e].T).astype(ml_dtypes.bfloat16)  # [P, dt_tiles, H]
        n_w1a = min(4, H // P)
        ca = n_w1a * P
        im = {
            "xt": xt,
            "w1a": np.ascontiguousarray(
                w1t[:, :, :ca].reshape(P, dt_tiles, n_w1a, P).transpose(0, 2, 1, 3)
            ),
            "w2t": _part3(w2[e].T).astype(ml_dtypes.bfloat16),
            "b1p": np.ascontiguousarray(b1[e].reshape(H // P, P).T),
        }
        if H > ca:
            im["w1b"] = np.ascontiguousarray(
                w1t[:, :, ca:]
                .reshape(P, dt_tiles, (H - ca) // 256, 256)
                .transpose(0, 2, 1, 3)
            )
        in_maps.append(im)

    # --- run the 8-core SPMD kernel ---
    nc = _get_nc(CAP, H, D, CHUNK)
    try:
        res = bass_utils.run_bass_kernel_spmd(nc, in_maps, core_ids=list(range(E)))
    except ImportError:
        # BASS_TRACE was requested but the axon NTFF hook isn't available.
        os.environ["BASS_NEVER_TRACE"] = "1"
        res = bass_utils.run_bass_kernel_spmd(nc, in_maps, core_ids=list(range(E)))
    LAST_RESULTS = res

    # --- weighted combine + b2 (unshard step) ---
    out = np.zeros((T, D), np.float32)
    for e in range(E):
        cnt = len(idx_e[e])
        yt = np.asarray(res.results[e]["yt"], np.float32)  # [128, D/128, CAP]
        y = yt.transpose(2, 1, 0).reshape(CAP, D)[:cnt]
        out[idx_e[e]] += wt_e[e][:, None] * (y + b2[e][None, :])

    for t_ov, e, w_ov in ovf:  # capacity overflow: exact host fallback
        hrow = _gelu_exact(xf[t_ov] @ w1[e].T + b1[e])
        out[t_ov] += w_ov * (hrow @ w2[e].T + b2[e])

    return out.reshape(B, S, D), sel.reshape(B, S, TOP_K)


# revision 19
# speedup vs baseline: 1.0416x; 1.0097x over previous
"""Expert-parallel MoE layer for one TRN2 chip (8 NeuronCores).

Problem: B=2, S=2048, D=1024, E=8, H=4*D, TOP_K=2 (nn_MoELayer, moe_routing).

Sharding strategy (expert-parallel, "all-to-all at shard time"):
  * The gate is tiny ([4096,1024]@[1024,8]) and runs on host in float64 while
    sharding.  float64 routing reproduces the fp32 reference's top-2 exactly:
    the minimum 2nd/3rd logit gap for these inputs is ~4.4e-4 while fp32
    accumulation noise is ~2e-6.
  * Expert e's (w1, b1, w2) are placed on core e.  The tokens routed to
    expert e (~1024 of 4096*2 slots, capacity CAP with zero padding) are
    gathered on host and shipped pre-transposed as xT[D, CAP].
  * Each core computes yT = w2 @ gelu(w1 @ xT + b1) with bf16 matmuls and
    fp32 PSUM accumulation; layouts are chosen so no on-device transpose is
    ever needed (mm1 produces h already contracted-major for mm2).
  * The softmax-weighted combine (+ per-expert b2) happens on host during the
    unshard/gather step.  Tokens above CAP (never for these shapes) fall back
    to an exact host computation.
"""

import math
import os

import numpy as np
import ml_dtypes

import concourse.bacc as bacc
import concourse.mybir as mybir
import concourse.tile as tile
from concourse import bass_utils
from concourse.bass import ds, ts
from concourse._compat import get_trn_type

P = 128
B, S, D, E, TOP_K = 2, 2048, 1024, 8, 2
H = 4 * D
T = B * S
CAP = 1088   # per-expert token capacity (max routed count is 1088 for these inputs)
CHUNK = 272  # token chunk per PSUM group; >230 keeps 97ns LDWEIGHTS hidden under the MM stream

BF16 = mybir.dt.bfloat16
F32 = mybir.dt.float32

GELU_FUNC = mybir.ActivationFunctionType.Gelu  # sim tests may swap (CoreSim lacks Gelu)

_BUILD_CACHE = {}
LAST_RESULTS = None  # BassKernelResults of the most recent device run (for profiling)


def _build(cap=CAP, h=H, d=D, chunk=CHUNK):
    """Build + compile the per-core Bass program (SPMD, one expert per core)."""
    dt_tiles, ht_tiles = d // P, h // P
    nc = bacc.Bacc(get_trn_type() or "TRN2", target_bir_lowering=False, debug=False)

    n_chunks = math.ceil(cap / chunk)
    # x and w1 are laid out in DRAM grouped by the DMA block that fetches
    # them (chunk-major for x, column-block-major for w1) so every DMA reads
    # long contiguous per-partition lines (>=2KB) at full HBM bandwidth while
    # still letting the first blocks arrive first.  w1's first 512 columns
    # ship as four 128-col blocks so the PE can start ~3us earlier; the rest
    # as 256-col blocks.
    n_w1a = min(4, h // P)
    n_w1b = (h - n_w1a * P) // 256
    x_d = nc.dram_tensor("xt", [P, n_chunks, dt_tiles, chunk], BF16, kind="ExternalInput")
    w1a_d = nc.dram_tensor("w1a", [P, n_w1a, dt_tiles, P], BF16, kind="ExternalInput")
    w1b_d = (
        nc.dram_tensor("w1b", [P, n_w1b, dt_tiles, 256], BF16, kind="ExternalInput")
        if n_w1b
        else None
    )
    w2_d = nc.dram_tensor("w2t", [P, ht_tiles, d], BF16, kind="ExternalInput")
    b1_d = nc.dram_tensor("b1p", [P, ht_tiles], F32, kind="ExternalInput")
    y_d = nc.dram_tensor("yt", [P, dt_tiles, cap], F32, kind="ExternalOutput")

    with tile.TileContext(nc) as tc:
        with (
            tc.tile_pool(name="wts", bufs=1) as wpool,
            tc.tile_pool(name="io", bufs=1) as ypool,
            tc.tile_pool(name="hts", bufs=2) as htpool,
            tc.tile_pool(name="ps1", bufs=4, space="PSUM") as ps1,
            tc.tile_pool(name="ps2", bufs=4, space="PSUM") as ps2,
        ):
            # DMA order shapes the pipeline lead-in: the first matmuls only
            # need x[chunk 0] plus w1's first column-block, so ship x chunk 0,
            # then w1 in column-blocks, then x chunk 1, then w2.  HBM can't
            # deliver w1+w2 (~17MB) inside one mm1 chunk, so the compute
            # below runs mm1(c0), mm1(c1) before mm2(c0) to buy the w2
            # transfer time.
            x_sb = wpool.tile([P, dt_tiles, cap], BF16)
            w1_sb = wpool.tile([P, dt_tiles, h], BF16)
            w2_sb = wpool.tile([P, ht_tiles, d], BF16)
            b1_sb = wpool.tile([P, ht_tiles], F32)

            def dma_x_chunk(ci):
                c0 = ci * chunk
                n = min(chunk, cap - c0)
                nc.sync.dma_start(
                    out=x_sb[:, :, ds(c0, n)], in_=x_d[:, ci, :, :n]
                )

            nc.sync.dma_start(out=b1_sb[:], in_=b1_d[:])
            # x chunk 0 in two k-halves so the first (m=0, k<half) matmuls
            # can issue before the whole chunk lands
            kh = max(1, dt_tiles // 2)
            nc.sync.dma_start(out=x_sb[:, :kh, ds(0, chunk)], in_=x_d[:, 0, :kh, :])
            nc.sync.dma_start(out=x_sb[:, kh:, ds(0, chunk)], in_=x_d[:, 0, kh:, :])
            for mb in range(n_w1a):
                nc.sync.dma_start(
                    out=w1_sb[:, :, ts(mb, P)], in_=w1a_d[:, mb, :, :]
                )
            for jb in range(n_w1b):
                nc.sync.dma_start(
                    out=w1_sb[:, :, ds(n_w1a * P + jb * 256, 256)],
                    in_=w1b_d[:, jb, :, :],
                )
            if n_chunks > 1:
                dma_x_chunk(1)
            for k in range(ht_tiles):
                nc.sync.dma_start(out=w2_sb[:, k, :], in_=w2_d[:, k, :])
            for ci in range(2, n_chunks):
                dma_x_chunk(ci)

            ht_tiles_sb = {}

            def mm1_chunk(ci):
                c0 = ci * chunk
                n = min(chunk, cap - c0)
                # mm1: hT[h_tile, tok] = sum_d w1t[d, h_tile] * xT[d, tok]
                ht_sb = htpool.tile([P, ht_tiles, chunk], BF16, tag="ht")
                ht_tiles_sb[ci] = ht_sb
                for m in range(ht_tiles):
                    ps = ps1.tile([P, chunk], F32, tag="ps1")
                    for k in range(dt_tiles):
                        nc.tensor.matmul(
                            ps[:, :n],
                            w1_sb[:, k, ts(m, P)],
                            x_sb[:, k, ds(c0, n)],
                            start=(k == 0),
                            stop=(k == dt_tiles - 1),
                        )
                    nc.scalar.activation(
                        ht_sb[:, m, :n],
                        ps[:, :n],
                        GELU_FUNC,
                        bias=b1_sb[:, m : m + 1],
                    )

            def mm2_chunk(ci):
                c0 = ci * chunk
                n = min(chunk, cap - c0)
                ht_sb = ht_tiles_sb.pop(ci)
                # mm2: yT[d_tile, tok] = sum_h w2t[h, d_tile] * hT[h, tok]
                y_sb = ypool.tile([P, dt_tiles, chunk], F32, tag="y")
                for m in range(dt_tiles):
                    ps = ps2.tile([P, chunk], F32, tag="ps2")
                    for k in range(ht_tiles):
                        nc.tensor.matmul(
                            ps[:, :n],
                            w2_sb[:, k, ts(m, P)],
                            ht_sb[:, k, :n],
                            start=(k == 0),
                            stop=(k == ht_tiles - 1),
                        )
                    nc.vector.tensor_copy(y_sb[:, m, :n], ps[:, :n])
                    nc.sync.dma_start(out=y_d[:, m, ds(c0, n)], in_=y_sb[:, m, :n])

            # software pipeline: two mm1 chunks in flight ahead of mm2
            mm1_chunk(0)
            if n_chunks > 1:
                mm1_chunk(1)
            mm2_chunk(0)
            for ci in range(2, n_chunks):
                mm1_chunk(ci)
                mm2_chunk(ci - 1)
            if n_chunks > 1:
                mm2_chunk(n_chunks - 1)

    nc.compile()
    return nc


def _get_nc(cap=CAP, h=H, d=D, chunk=CHUNK):
    key = (cap, h, d, chunk)
    if key not in _BUILD_CACHE:
        _BUILD_CACHE[key] = _build(*key)
    return _BUILD_CACHE[key]


def _part3(mat_t):
    """[Dim, N] (Dim multiple of 128) -> partition-major [128, Dim/128, N]."""
    dim, n = mat_t.shape
    return np.ascontiguousarray(mat_t.reshape(dim // P, P, n).transpose(1, 0, 2))


def _gelu_exact(v):
    from scipy.special import erf

    return 0.5 * v * (1.0 + erf(v / np.sqrt(2.0)))


def kernel(x, gate_w, gate_b, w1, b1, w2, b2):
    global LAST_RESULTS
    xf = np.asarray(x, np.float32).reshape(T, D)
    gate_w = np.asarray(gate_w, np.float32)
    gate_b = np.asarray(gate_b, np.float32)
    w1 = np.asarray(w1, np.float32)
    b1 = np.asarray(b1, np.float32)
    w2 = np.asarray(w2, np.float32)
    b2 = np.asarray(b2, np.float32)

    # --- host gate + top-2 routing (float64: robust vs fp32 rounding) ---
    logits = xf.astype(np.float64) @ gate_w.T.astype(np.float64) + gate_b
    sel = np.argsort(-logits, axis=1, kind="stable")[:, :TOP_K].astype(np.int32)
    lsel = np.take_along_axis(logits, sel.astype(np.int64), axis=1)
    ex = np.exp(lsel - lsel.max(axis=1, keepdims=True))
    wts = (ex / ex.sum(axis=1, keepdims=True)).astype(np.float32)

    # --- gather tokens per expert (shard step) ---
    idx_e, wt_e, ovf = [], [], []
    in_maps = []
    xT = np.ascontiguousarray(xf.T)  # [D, T]
    for e in range(E):
        m0 = sel[:, 0] == e
        m1 = sel[:, 1] == e
        idx = np.nonzero(m0 | m1)[0]
        wt = np.where(m0[idx], wts[idx, 0], wts[idx, 1]).astype(np.float32)
        if len(idx) > CAP:
            for t_ov, w_ov in zip(idx[CAP:], wt[CAP:]):
                ovf.append((int(t_ov), e, float(w_ov)))
            idx, wt = idx[:CAP], wt[:CAP]
        idx_e.append(idx)
        wt_e.append(wt)

        n_chunks = CAP // CHUNK
        dt_tiles = D // P
        xt = np.zeros((P, dt_tiles, CAP), ml_dtypes.bfloat16)
        xt[:, :, : len(idx)] = _part3(xT[:, idx]).astype(ml_dtypes.bfloat16)
        # group by DMA block: x chunk-major, w1 column-block-major (contiguous lines)
        xt = np.ascontiguousarray(
            xt.reshape(P, dt_tiles, n_chunks, CHUNK).transpose(0, 2, 1, 3)
        )
        w1t = _part3(w1[e].T).astype(ml_dtypes.bfloat16)  # [P, dt_tiles, H]
        n_w1a = min(4, H // P)
        ca = n_w1a * P
        im = {
            "xt": xt,
            "w1a": np.ascontiguousarray(
                w1t[:, :, :ca].reshape(P, dt_tiles, n_w1a, P).transpose(0, 2, 1, 3)
            ),
            "w2t": _part3(w2[e].T).astype(ml_dtypes.bfloat16),
            "b1p": np.ascontiguousarray(b1[e].reshape(H // P, P).T),
        }
        if H > ca:
            im["w1b"] = np.ascontiguousarray(
                w1t[:, :, ca:]
                .reshape(P, dt_tiles, (H - ca) // 256, 256)
                .transpose(0, 2, 1, 3)
            )
        in_maps.append(im)

    # --- run the 8-core SPMD kernel ---
    nc = _get_nc(CAP, H, D, CHUNK)
    try:
        res = bass_utils.run_bass_kernel_spmd(nc, in_maps, core_ids=list(range(E)))
    except ImportError:
        # BASS_TRACE was requested but the axon NTFF hook isn't available.
        os.environ["BASS_NEVER_TRACE"] = "1"
        res = bass_utils.run_bass_kernel_spmd(nc, in_maps, core_ids=list(range(E)))
    LAST_RESULTS = res

    # --- weighted combine + b2 (unshard step) ---
    out = np.zeros((T, D), np.float32)
    for e in range(E):
        cnt = len(idx_e[e])
        yt = np.asarray(res.results[e]["yt"], np.float32)  # [128, D/128, CAP]
        y = yt.transpose(2, 1, 0).reshape(CAP, D)[:cnt]
        out[idx_e[e]] += wt_e[e][:, None] * (y + b2[e][None, :])

    for t_ov, e, w_ov in ovf:  # capacity overflow: exact host fallback
        hrow = _gelu_exact(xf[t_ov] @ w1[e].T + b1[e])
        out[t_ov] += w_ov * (hrow @ w2[e].T + b2[e])

    return out.reshape(B, S, D), sel.reshape(B, S, TOP_K)
